# revision 1
# baseline (speedup 1.0000x reference)
"""Trainium2 Bass kernel for nn_MultiHeadAttention_61091614818698.

Contract: kernel(**inputs) takes the FULL unsharded inputs
(x [2,2048,1024], Wq/Wk/Wv [16,1024,64], bq/bk/bv [16,64], Wo [1024,1024],
bo [1024]) and returns the FULL output [2,2048,1024].

Strategy: tensor-parallel over heads -- 2 heads per NeuronCore on 8 cores.
Each core computes Q^T/K^T (heads stacked on partitions), V via PE
transpose, scoresT with the two K=64 head matmuls packed into PE row
halves (tile_position), softmax without max-subtraction (scores are
O(1)-bounded), denominators from an appended ones-column in the V
matmul, 1/d = exp(-ln d) broadcast via a K=1 matmul, and a partial
output projection against its 128-column slice of Wo^T.  The host sums
the 8 partial projections and adds bo (grand reduction over head dims).
Matmuls run in float32r (full PE rate; ~1e-4 relative rounding).
"""
import numpy as np
import concourse.bass as bass
import concourse.mybir as mybir
import concourse.tile as tile
from concourse import bacc

F32 = mybir.dt.float32
F32R = mybir.dt.float32r
AF = mybir.ActivationFunctionType
ALU = mybir.AluOpType


class _Bacc(bacc.Bacc):
    """Bacc that pins Exp and Ln to the combined natural_log_exp table set,
    avoiding a per-chunk exp<->ln table-load thrash (~5.3us per q-chunk)."""

    def insert_act_table_loads(self):
        import bass_rust as _br
        from concourse.hw_specs import get_activation_tables
        has_activation = any(
            type(i).__name__ == "InstActivation"
            for b in self.main_func.blocks for i in b.instructions)
        if not has_activation:
            return
        tables = []
        for name, funcs in get_activation_tables(self.m.arch).items():
            if name != "natural_log_exp_and_others":
                funcs = set()
            tables.append((name, funcs))
        _br.insert_act_table_loads(self, tables)


def build_nc(B=2, S=2048, D=1024, HPC=2, use_f32r=True, n_cores=8, repeat=1, phases=(1, 2)):
    T = B * S
    TCH = T // 512          # token chunks for projections
    DC = D // 128           # contraction chunks
    QCW = min(512, S)       # q-chunk width
    QC = S // QCW           # q chunks per batch
    ST = S // 128           # s tiles per batch
    G = T // 128            # global token tiles
    MD = F32R if use_f32r else F32

    nc = _Bacc("TRN2", target_bir_lowering=False, debug=False,
               num_devices=n_cores)
    xt = nc.dram_tensor("xt", [D, T], MD, kind="ExternalInput").ap()
    wq = nc.dram_tensor("wq", [128, D], MD, kind="ExternalInput").ap()
    wk = nc.dram_tensor("wk", [128, D], MD, kind="ExternalInput").ap()
    wv = nc.dram_tensor("wv", [128, D], MD, kind="ExternalInput").ap()
    bq = nc.dram_tensor("bq", [128, 1], F32, kind="ExternalInput").ap()
    bk = nc.dram_tensor("bk", [128, 1], F32, kind="ExternalInput").ap()
    bv = nc.dram_tensor("bv", [128, 1], F32, kind="ExternalInput").ap()
    wo = nc.dram_tensor("wo", [128, D], MD, kind="ExternalInput").ap()
    ident = nc.dram_tensor("ident", [128, 128], MD, kind="ExternalInput").ap()
    ones64 = nc.dram_tensor("ones64", [1, 64], MD, kind="ExternalInput").ap()
    onescol = nc.dram_tensor("onescol", [128, 1], MD, kind="ExternalInput").ap()
    po = nc.dram_tensor("po", [T, D], F32, kind="ExternalOutput").ap()

    with tile.TileContext(nc) as tc:
        with tc.tile_pool(name="singles", bufs=1) as singles:
            wq_sb = singles.tile([128, D], MD, tag="wq")
            wk_sb = singles.tile([128, D], MD, tag="wk")
            wv_sb = singles.tile([128, D], MD, tag="wv")
            wo_sb = singles.tile([128, D], MD, tag="wo")
            bq_sb = singles.tile([128, 1], F32, tag="bq")
            bk_sb = singles.tile([128, 1], F32, tag="bk")
            bv_sb = singles.tile([128, 1], F32, tag="bv")
            id_sb = singles.tile([128, 128], MD, tag="id")
            o64_sb = singles.tile([1, 64], MD, tag="o64")
            qt2 = singles.tile([128, T], MD, tag="qt2")
            kt2 = singles.tile([128, T], MD, tag="kt2")
            vs = singles.tile([128, G * 2 * 65], MD, tag="vs")
            vs_r = vs[:].rearrange("p (g n) -> p g n", n=65)

            for dst, src in ((wq_sb, wq), (wk_sb, wk), (wv_sb, wv),
                             (wo_sb, wo), (bq_sb, bq), (bk_sb, bk),
                             (bv_sb, bv), (id_sb, ident), (o64_sb, ones64)):
                nc.sync.dma_start(out=dst[:], in_=src[:])
            ones_bcast = bass.AP(
                tensor=onescol.tensor, offset=onescol.offset,
                ap=[list(onescol.ap[0]), [0, G * 2], list(onescol.ap[1])])
            nc.sync.dma_start(out=vs_r[:, :, 64:65], in_=ones_bcast)

            for _rep in range(repeat):
                # ---- phase 1: projections ----
                if 1 not in phases:
                    continue
                with tc.tile_pool(name="xt_pool", bufs=16) as xt_pool, \
                     tc.tile_pool(name="vtmp", bufs=3) as vtmp_pool, \
                     tc.tile_pool(name="ps_qkv", bufs=2, space="PSUM") as ps_qkv, \
                     tc.tile_pool(name="ps_tr", bufs=2, space="PSUM") as ps_tr:
                    for tch in range(TCH):
                        tsl = bass.ts(tch, 512)
                        xx = []
                        for dc in range(DC):
                            xtile = xt_pool.tile([128, 512], MD, tag="xt")
                            nc.sync.dma_start(
                                out=xtile[:],
                                in_=xt[dc * 128:(dc + 1) * 128, tsl])
                            xx.append(xtile)
                        pq = ps_qkv.tile([128, 512], F32, tag="pq")
                        pk = ps_qkv.tile([128, 512], F32, tag="pk")
                        pv = ps_qkv.tile([128, 512], F32, tag="pv")
                        for dc in range(DC):
                            dsl = bass.ts(dc, 128)
                            st_, sp_ = dc == 0, dc == DC - 1
                            nc.tensor.matmul(pq[:], wq_sb[:, dsl], xx[dc][:],
                                             start=st_, stop=sp_)
                            nc.tensor.matmul(pk[:], wk_sb[:, dsl], xx[dc][:],
                                             start=st_, stop=sp_)
                            nc.tensor.matmul(pv[:], wv_sb[:, dsl], xx[dc][:],
                                             start=st_, stop=sp_)
                        nc.vector.tensor_scalar_add(qt2[:, tsl], pq[:], bq_sb[:])
                        nc.vector.tensor_scalar_add(kt2[:, tsl], pk[:], bk_sb[:])
                        vt = vtmp_pool.tile([128, 512], MD, tag="vt")
                        nc.vector.tensor_scalar_add(vt[:], pv[:], bv_sb[:])
                        for i in range(4):
                            g = tch * 4 + i
                            for h in range(HPC):
                                ptr = ps_tr.tile([128, 64], MD, tag="tr")
                                nc.tensor.transpose(
                                    ptr[:],
                                    vt[h * 64:(h + 1) * 64, bass.ts(i, 128)],
                                    id_sb[h * 64:h * 64 + 64,
                                          h * 64:h * 64 + 64])
                                nc.vector.tensor_copy(
                                    vs_r[:, g * 2 + h, 0:64], ptr[:])

                # ---- phase 2 (attention) + phase 3 (out-projection) ----
                if 2 not in phases:
                    nc.sync.dma_start(out=po[0:128, 0:D], in_=qt2[:, 0:D].bitcast(F32))
                    continue
                with tc.tile_pool(name="ps_s", bufs=4, space="PSUM") as ps_s, \
                     tc.tile_pool(name="ps_o", bufs=2, space="PSUM") as ps_o, \
                     tc.tile_pool(name="ps_m", bufs=2, space="PSUM") as ps_m, \
                     tc.tile_pool(name="expp", bufs=4) as expp, \
                     tc.tile_pool(name="rbp", bufs=2) as rbp, \
                     tc.tile_pool(name="lnp", bufs=2) as lnp, \
                     tc.tile_pool(name="o2tp", bufs=B * QC) as o2tp, \
                     tc.tile_pool(name="outp", bufs=4) as outp:
                    for b in range(B):
                        for qc in range(QC):
                            qsl = bass.ds(b * S + qc * QCW, QCW)
                            o2t = o2tp.tile([128, QCW], MD, tag="o2t")
                            oacc = []
                            for _h in range(HPC):
                                oacc_t = ps_o.tile([128, QCW], F32, tag="oacc")
                                oacc.append(oacc_t)

                            def emit_scores(st):
                                ssl = bass.ds(b * S + st * 128, 128)
                                out = []
                                for h in range(HPC):
                                    hp = h * 64
                                    ps = ps_s.tile([128, QCW], F32, tag="ps")
                                    nc.tensor.matmul(
                                        ps[:], kt2[hp:hp + 64, ssl],
                                        qt2[hp:hp + 64, qsl],
                                        start=True, stop=True,
                                        tile_position=(hp, 0),
                                        skip_group_check=True)
                                    out.append(ps)
                                return out

                            # software pipeline: keep two s-tiles of scores in
                            # flight so the PE never stalls on ACT's exp.
                            pend = {0: emit_scores(0)}
                            if ST > 1:
                                pend[1] = emit_scores(1)
                            for st in range(ST):
                                g = b * ST + st
                                pss = pend.pop(st)
                                for h in range(HPC):
                                    e = expp.tile([128, QCW], MD, tag="e")
                                    nc.scalar.activation(e[:], pss[h], AF.Exp,
                                                         scale=0.125)
                                    nc.tensor.matmul(
                                        oacc[h][0:65, :],
                                        vs_r[:, g * 2 + h, :], e[:],
                                        start=(st == 0), stop=(st == ST - 1),
                                        skip_group_check=True)
                                if st + 2 < ST:
                                    pend[st + 2] = emit_scores(st + 2)
                            for h in range(HPC):
                                o = oacc[h]
                                ln_t = lnp.tile([1, QCW], MD, tag="ln")
                                nc.scalar.activation(ln_t[:], o[64:65, :], AF.Ln)
                                pb = ps_m.tile([64, QCW], F32, tag="m")
                                nc.tensor.matmul(pb[:], o64_sb[:], ln_t[:],
                                                 start=True, stop=True,
                                                 skip_group_check=True)
                                rb = rbp.tile([64, QCW], F32, tag="rb")
                                nc.scalar.activation(rb[:], pb[:], AF.Exp,
                                                     scale=-1.0)
                                nc.vector.tensor_tensor(
                                    out=o2t[h * 64:(h + 1) * 64, :],
                                    in0=o[0:64, :], in1=rb[:], op=ALU.mult)
                            for i in range(QCW // 128):
                                gt = b * ST + qc * (QCW // 128) + i
                                for ec in range(D // 512):
                                    pp = ps_m.tile([128, 512], F32, tag="m")
                                    nc.tensor.matmul(
                                        pp[:], o2t[:, bass.ts(i, 128)],
                                        wo_sb[:, bass.ts(ec, 512)],
                                        start=True, stop=True,
                                        skip_group_check=True)
                                    ot = outp.tile([128, 512], F32, tag="ot")
                                    nc.vector.tensor_copy(ot[:], pp[:])
                                    nc.sync.dma_start(
                                        out=po[gt * 128:(gt + 1) * 128,
                                               bass.ts(ec, 512)],
                                        in_=ot[:])
    nc.compile()
    return nc


def host_inputs(x, Wq, bqv, Wk, bkv, Wv, bvv, Wo, n_cores=8, hpc=2):
    """Build per-core input maps. x:[B,S,D]; Wq/Wk/Wv:[H,D,64]; b*:[H,64]; Wo:[D,D]."""
    B, S, D = x.shape
    T = B * S
    xt = np.ascontiguousarray(x.reshape(T, D).T).astype(np.float32)
    ident = np.eye(128, dtype=np.float32)
    ones64 = np.ones((1, 64), dtype=np.float32)
    wot = np.ascontiguousarray(Wo.T).astype(np.float32)

    def wpack(W, c):
        W2 = np.concatenate([W[hpc * c + j] for j in range(hpc)], axis=1)
        return np.ascontiguousarray(
            W2.reshape(D // 128, 128, 128).transpose(1, 0, 2).reshape(128, D))

    def bpack(bb, c):
        return np.concatenate([bb[hpc * c + j] for j in range(hpc)]
                              ).reshape(128, 1).astype(np.float32)

    maps = []
    for c in range(n_cores):
        maps.append({
            "xt": xt,
            "wq": wpack(Wq, c), "wk": wpack(Wk, c), "wv": wpack(Wv, c),
            "bq": bpack(bqv, c), "bk": bpack(bkv, c), "bv": bpack(bvv, c),
            "wo": np.ascontiguousarray(wot[c * 128:(c + 1) * 128, :]),
            "ident": ident, "ones64": ones64,
            "onescol": np.ones((128, 1), dtype=np.float32),
        })
    return maps


class Runner:
    """Compile once, run many times through the PJRT/axon path."""

    def __init__(self, nc, n_cores=8):
        import jax
        import numpy as _np
        from jax.sharding import Mesh, PartitionSpec
        from jax.experimental.shard_map import shard_map
        from concourse import bass2jax, mybir as _mybir
        bass2jax.install_neuronx_cc_hook()
        self.jax = jax
        self.n_cores = n_cores
        partition_name = (nc.partition_id_tensor.name
                          if nc.partition_id_tensor else None)
        self.partition_name = partition_name
        in_names, out_names, out_avals, zero_outs = [], [], [], []
        for alloc in nc.m.functions[0].allocations:
            if not isinstance(alloc, _mybir.MemoryLocationSet):
                continue
            name = alloc.memorylocations[0].name
            if alloc.kind == "ExternalInput":
                if name != partition_name:
                    in_names.append(name)
            elif alloc.kind == "ExternalOutput":
                out_names.append(name)
                shape = tuple(alloc.tensor_shape)
                dtype = _mybir.dt.np(alloc.dtype)
                out_avals.append(jax.core.ShapedArray(shape, dtype))
                zero_outs.append((shape, dtype))
        self.in_names, self.out_names = list(in_names), list(out_names)
        self.out_avals, self.zero_shapes = out_avals, zero_outs
        n_params, n_outs = len(in_names), len(out_names)
        self.n_params = n_params
        all_names = in_names + out_names
        if partition_name is not None:
            all_names = all_names + [partition_name]

        def _body(*args):
            operands = list(args)
            if partition_name is not None:
                operands.append(bass2jax.partition_id_tensor())
            outs = bass2jax._bass_exec_p.bind(
                *operands,
                out_avals=tuple(out_avals),
                in_names=tuple(all_names),
                out_names=tuple(out_names),
                lowering_input_output_aliases=(),
                sim_require_finite=True,
                sim_require_nnan=True,
                nc=nc,
            )
            return tuple(outs)

        devices = jax.devices()[:n_cores]
        self.mesh = Mesh(_np.asarray(devices), ("core",))
        self.pspec = PartitionSpec("core")
        in_specs = (self.pspec,) * (n_params + n_outs)
        out_specs = (self.pspec,) * n_outs
        self.donate = tuple(range(n_params, n_params + n_outs))
        self.fn = jax.jit(
            shard_map(_body, mesh=self.mesh, in_specs=in_specs,
                      out_specs=out_specs, check_rep=False),
            donate_argnums=self.donate, keep_unused=True)

    def stage_inputs(self, in_maps):
        import numpy as _np
        from jax.sharding import NamedSharding
        sh = NamedSharding(self.mesh, self.pspec)
        staged = []
        for name in self.in_names:
            g = _np.concatenate([_np.asarray(m[name]) for m in in_maps],
                                axis=0)
            staged.append(self.jax.device_put(g, sh))
        return staged

    def make_zeros(self):
        import numpy as _np
        from jax.sharding import NamedSharding
        sh = NamedSharding(self.mesh, self.pspec)
        return [self.jax.device_put(
                    _np.zeros((self.n_cores * s[0], *s[1:]), d), sh)
                for (s, d) in self.zero_shapes]

    def run(self, staged_in, zeros):
        return self.fn(*staged_in, *zeros)

    def results(self, outs):
        import numpy as _np
        res = []
        for c in range(self.n_cores):
            d = {}
            for i, name in enumerate(self.out_names):
                a = self.out_avals[i]
                d[name] = _np.asarray(outs[i]).reshape(
                    self.n_cores, *a.shape)[c]
            res.append(d)
        return res


_STATE = {}


def _get_runner():
    if "runner" not in _STATE:
        nc = build_nc(B=2, S=2048, D=1024, HPC=2, use_f32r=True, n_cores=8,
                      repeat=1, phases=(1, 2))
        _STATE["runner"] = Runner(nc, n_cores=8)
    return _STATE["runner"]


def kernel(x, Wq, bq, Wk, bk, Wv, bv, Wo, bo):
    import numpy as _np
    x = _np.asarray(x, dtype=_np.float32)
    Wq = _np.asarray(Wq, dtype=_np.float32)
    bq_ = _np.asarray(bq, dtype=_np.float32)
    Wk = _np.asarray(Wk, dtype=_np.float32)
    bk_ = _np.asarray(bk, dtype=_np.float32)
    Wv = _np.asarray(Wv, dtype=_np.float32)
    bv_ = _np.asarray(bv, dtype=_np.float32)
    Wo = _np.asarray(Wo, dtype=_np.float32)
    bo_ = _np.asarray(bo, dtype=_np.float32)
    B, S, D = x.shape
    r = _get_runner()
    maps = host_inputs(x, Wq, bq_, Wk, bk_, Wv, bv_, Wo)
    staged = r.stage_inputs(maps)
    outs = r.run(staged, r.make_zeros())
    res = r.results(outs)
    acc = _np.zeros((B * S, D), dtype=_np.float32)
    for c in range(8):
        acc += res[c]["po"]
    return (acc.reshape(B, S, D) + bo_).astype(_np.float32)

